# revision 5
# baseline (speedup 1.0000x reference)
"""Segment-max kernel for Trainium2 (8 NeuronCores, SPMD).

Strategy (data-parallel, per sharding hint):
  - Shard embeddings/study_indexes along N across 8 cores (62500 rows each).
  - Host: per core, sort the shard's rows by segment id and pad each
    segment's run to a multiple of 128 rows (repeating a row of the same
    segment), so every 128-row tile is segment-pure.  Lay the sorted
    rows out pre-swizzled as [128, NT*256] so each SBUF tile load is a
    contiguous 32KB-per-partition DMA.
  - Device: stream tiles at full HBM bandwidth; for each 128-row tile,
    transpose the two 128-feature halves onto PSUM with the TensorEngine
    and reduce_max across rows with the VectorEngine, producing a
    [256]-wide partial max per tile.
  - Host: combine per-tile partials by segment (tiles of a segment are
    consecutive), then max across cores (the "all-reduce with max").
"""

import sys

sys.path.insert(0, "/opt/trn_rl_repo")

import numpy as np

import concourse.bacc as bacc
import concourse.bass as bass
import concourse.mybir as mybir
from concourse.masks import make_identity

P = 128               # SBUF partitions
D = 256               # embedding dim
CHUNK_TILES = 32      # tiles per DMA chunk (4MB)
N_CORES = 8

_NC_CACHE = {}


def build_nc(NT):
    """Bass program: NT segment-pure 128-row tiles -> per-tile max partials.

    Inputs : emb   [128, NT*256] f32  (tile t = columns [t*256,(t+1)*256);
                                       partition p holds row p of the tile)
    Outputs: parts [128, 2*NT]   f32  (col t: max of tile t, features 0-127
                                       in partitions; col NT+t: features
                                       128-255)
    """
    assert NT % CHUNK_TILES == 0
    NCHUNK = NT // CHUNK_TILES
    NGROUP = NCHUNK * 8           # 4-tile groups
    f32 = mybir.dt.float32

    nc = bacc.Bacc("TRN2")
    emb = nc.declare_dram_parameter("emb", [P, NT * D], f32, isOutput=False)
    parts = nc.declare_dram_parameter("parts", [P, 2 * NT], f32, isOutput=True)

    with (
        nc.Block() as block,
        nc.sbuf_tensor("chunk0", [P, CHUNK_TILES * D], f32) as chunk0,
        nc.sbuf_tensor("chunk1", [P, CHUNK_TILES * D], f32) as chunk1,
        nc.sbuf_tensor("ident", [P, P], f32) as ident,
        nc.sbuf_tensor("partials", [P, 2 * NT], f32) as partials,
        nc.psum_tensor([P, 512], f32) as ps00,   # half0, buffer q=0
        nc.psum_tensor([P, 512], f32) as ps01,   # half0, q=1
        nc.psum_tensor([P, 512], f32) as ps10,   # half1, q=0
        nc.psum_tensor([P, 512], f32) as ps11,   # half1, q=1
        nc.semaphore("ini") as ini,
        nc.semaphore("ld0") as ld0,
        nc.semaphore("ld1") as ld1,
        nc.semaphore("st") as st,
        nc.semaphore("tp") as tp,
        nc.semaphore("vr") as vr,
    ):
        lds = [ld0, ld1]
        chunks = [chunk0, chunk1]
        ph0 = [ps00, ps01]
        ph1 = [ps10, ps11]

        @block.gpsimd
        def _(gpsimd: bass.BassGpSimd):
            nc.gpsimd.memset(ident[:], 0.0).then_inc(ini, 1)
            gpsimd.wait_ge(ini, 1)
            nc.gpsimd.affine_select(
                out=ident[:],
                in_=ident[:],
                compare_op=mybir.AluOpType.not_equal,
                fill=1.0,
                base=0,
                pattern=[[-1, P]],
                channel_multiplier=1,
            ).then_inc(ini, 1)

        @block.sync
        def _(sync: bass.BassEngine):
            for c in range(NCHUNK):
                if c >= 2:
                    # chunk buffer c%2 is free once all 64 transposes of
                    # chunk c-2 have read it
                    sync.wait_ge(tp, 64 * (c - 1))
                sync.dma_start(
                    chunks[c % 2][:],
                    emb[:, c * CHUNK_TILES * D : (c + 1) * CHUNK_TILES * D],
                ).then_inc(lds[c % 2], 16)
            sync.wait_ge(vr, 2 * NGROUP)
            sync.dma_start(parts[:], partials[:]).then_inc(st, 16)
            sync.wait_ge(st, 16)

        @block.tensor
        def _(tensor: bass.BassEngine):
            tensor.wait_ge(ini, 2)
            for G in range(NGROUP):
                c, g = divmod(G, 8)
                b = c % 2
                q = G % 2
                if g == 0:
                    tensor.wait_ge(lds[b], 16 * (c // 2 + 1))
                if G >= 2:
                    # psum pair q free once group G-2's reduces are done
                    tensor.wait_ge(vr, 2 * (G - 1))
                for i in range(4):
                    tl = g * 4 + i
                    nc.tensor.transpose(
                        out=ph0[q][:, i * P : (i + 1) * P],
                        in_=chunks[b][:, tl * D : tl * D + P],
                        identity=ident[:],
                    ).then_inc(tp, 1)
                    nc.tensor.transpose(
                        out=ph1[q][:, i * P : (i + 1) * P],
                        in_=chunks[b][:, tl * D + P : (tl + 1) * D],
                        identity=ident[:],
                    ).then_inc(tp, 1)

        @block.vector
        def _(vector: bass.BassEngine):
            for G in range(NGROUP):
                c, g = divmod(G, 8)
                q = G % 2
                t0 = c * CHUNK_TILES + g * 4
                vector.wait_ge(tp, 8 * (G + 1))
                nc.vector.reduce_max(
                    partials[:, t0 : t0 + 4],
                    ph0[q][:].rearrange("p (t r) -> p t r", r=P),
                    axis=mybir.AxisListType.X,
                ).then_inc(vr, 1)
                nc.vector.reduce_max(
                    partials[:, NT + t0 : NT + t0 + 4],
                    ph1[q][:].rearrange("p (t r) -> p t r", r=P),
                    axis=mybir.AxisListType.X,
                ).then_inc(vr, 1)

    nc.compile()
    return nc


def _plan_core(idx_c, S):
    """Per-core tile plan: returns (padded_row_order, tile_seg) where
    padded_row_order has length 128*ntiles and every 128-row tile is
    segment-pure.  tile_seg[t] = segment id of tile t."""
    order = np.argsort(idx_c, kind="stable")
    counts = np.bincount(idx_c, minlength=S)
    starts = np.zeros(S + 1, np.int64)
    np.cumsum(counts, out=starts[1:])
    rows_parts = []
    tile_seg_parts = []
    for s in range(S):
        cnt = int(counts[s])
        if cnt == 0:
            continue
        rows = order[starts[s] : starts[s + 1]]
        ntile = (cnt + P - 1) // P
        pad = ntile * P - cnt
        if pad:
            rows = np.concatenate([rows, np.full(pad, rows[0], np.int64)])
        rows_parts.append(rows)
        tile_seg_parts.append(np.full(ntile, s, np.int64))
    return np.concatenate(rows_parts), np.concatenate(tile_seg_parts)


def kernel(embeddings, study_indexes, num_segments):
    from concourse.bass_utils import run_bass_kernel_spmd

    emb = np.ascontiguousarray(np.asarray(embeddings, dtype=np.float32))
    idx = np.asarray(study_indexes).astype(np.int64)
    S = int(num_segments)
    N = emb.shape[0]
    Nc = N // N_CORES

    plans = []
    for c in range(N_CORES):
        plans.append(_plan_core(idx[c * Nc : (c + 1) * Nc], S))
    ntiles_req = max(len(rows) // P for rows, _ in plans)
    NT = -(-ntiles_req // CHUNK_TILES) * CHUNK_TILES

    nc = _NC_CACHE.get(NT)
    if nc is None:
        nc = _NC_CACHE[NT] = build_nc(NT)

    in_maps = []
    for c in range(N_CORES):
        rows, _ = plans[c]
        shard = emb[c * Nc : (c + 1) * Nc]
        full_rows = np.empty(NT * P, np.int64)
        full_rows[: len(rows)] = rows
        full_rows[len(rows) :] = 0  # filler tiles, ignored on combine
        # [NT*128, 256] -> [128, NT*256]: tile t, row-in-tile p -> (p, t)
        arr = shard[full_rows].reshape(NT, P, D).transpose(1, 0, 2).reshape(P, NT * D)
        in_maps.append({"emb": np.ascontiguousarray(arr)})

    res = run_bass_kernel_spmd(nc, in_maps, list(range(N_CORES)))
    global _LAST_RESULT
    _LAST_RESULT = res

    out = np.full((S, D), -np.inf, dtype=np.float32)
    for c in range(N_CORES):
        parts = res.results[c]["parts"]     # [128, 2*NT]
        _, tile_seg = plans[c]
        nt_real = len(tile_seg)
        p0 = parts[:, :nt_real]             # [128 feats0-127, tiles]
        p1 = parts[:, NT : NT + nt_real]    # [128 feats128-255, tiles]
        bounds = np.nonzero(np.diff(tile_seg))[0] + 1
        seg_ids = tile_seg[np.concatenate([[0], bounds])]
        m0 = np.maximum.reduceat(p0, np.concatenate([[0], bounds]), axis=1)
        m1 = np.maximum.reduceat(p1, np.concatenate([[0], bounds]), axis=1)
        for k, s in enumerate(seg_ids):
            np.maximum(out[s, :P], m0[:, k], out=out[s, :P])
            np.maximum(out[s, P:], m1[:, k], out=out[s, P:])
    return out


# revision 8
# speedup vs baseline: 1.3216x; 1.3216x over previous
"""Segment-max kernel for Trainium2 (8 NeuronCores, SPMD).

Strategy (data-parallel, per the sharding hint):
  - Shard embeddings/study_indexes along N across 8 cores (62500 rows each).
  - Host: per core, sort the shard's rows by segment id (indexes only) and
    lay the sorted rows out pre-swizzled as [128, NT*256] so each SBUF tile
    load is a contiguous per-partition DMA at full HBM bandwidth.
  - Device: stream 128-row tiles; for each tile, transpose the two
    128-feature halves onto PSUM (TensorEngine) and reduce_max across the
    rows (VectorEngine), producing a [256]-wide partial max per tile.
  - Host: pure tiles (single segment) combine via their device partials;
    the ~63 boundary tiles per core are re-reduced from the raw rows.
    Finally max across cores (the "all-reduce with max").
"""

import sys

sys.path.insert(0, "/opt/trn_rl_repo")

from contextlib import ExitStack

import numpy as np

import concourse.bacc as bacc
import concourse.bass as bass
import concourse.mybir as mybir

P = 128               # SBUF partitions
D = 256               # embedding dim
CHUNK_TILES = 32      # tiles per DMA chunk (4MB)
NBUF = 4              # chunk buffer depth
NPSUM = 4             # psum (bank-pair) pipeline depth
N_CORES = 8

_NC_CACHE = {}


def build_nc(NT):
    """Bass program: NT 128-row tiles -> per-tile max partials.

    Inputs : emb   [128, NT*256] f32  (tile t = columns [t*256,(t+1)*256);
                                       partition p holds row p of the tile)
    Outputs: parts [128, 2*NT]   f32  (col t: max of tile t, features 0-127
                                       in partitions; col NT+t: features
                                       128-255)
    """
    f32 = mybir.dt.float32
    chunk_sizes = []
    left = NT
    while left > 0:
        chunk_sizes.append(min(CHUNK_TILES, left))
        left -= CHUNK_TILES

    # group schedule: (chunk, buf, psum_q, tile0_global, tile0_in_chunk, k)
    groups = []
    t_global = 0
    for c, csz in enumerate(chunk_sizes):
        for g0 in range(0, csz, 4):
            k = min(4, csz - g0)
            groups.append((c, c % NBUF, len(groups) % NPSUM, t_global + g0, g0, k))
        t_global += csz
    NGROUP = len(groups)

    tp_after_group = list(np.cumsum([2 * g[5] for g in groups]))
    groups_per_chunk = [(csz + 3) // 4 for csz in chunk_sizes]
    chunk_last_group = list(np.cumsum(groups_per_chunk) - 1)
    tp_after_chunk = [tp_after_group[i] for i in chunk_last_group]
    vr_after_group = [2 * (i + 1) for i in range(NGROUP)]

    nc = bacc.Bacc("TRN2")
    emb = nc.declare_dram_parameter("emb", [P, NT * D], f32, isOutput=False)
    parts = nc.declare_dram_parameter("parts", [P, 2 * NT], f32, isOutput=True)

    with (
        nc.Block() as block,
        nc.sbuf_tensor("ident", [P, P], f32) as ident,
        nc.sbuf_tensor("partials", [P, 2 * NT], f32) as partials,
        nc.semaphore("ini") as ini,
        nc.semaphore("st") as st,
        nc.semaphore("tp") as tp,
        nc.semaphore("vr") as vr,
        ExitStack() as stack,
    ):
        chunks = [
            stack.enter_context(
                nc.sbuf_tensor(f"chunk{i}", [P, CHUNK_TILES * D], f32)
            )
            for i in range(NBUF)
        ]
        ph0 = [
            stack.enter_context(nc.psum_tensor(f"ph0_{i}", [P, 512], f32))
            for i in range(NPSUM)
        ]
        ph1 = [
            stack.enter_context(nc.psum_tensor(f"ph1_{i}", [P, 512], f32))
            for i in range(NPSUM)
        ]
        lds = [stack.enter_context(nc.semaphore(f"ld{i}")) for i in range(NBUF)]

        @block.gpsimd
        def _(gpsimd: bass.BassGpSimd):
            nc.gpsimd.memset(ident[:], 0.0).then_inc(ini, 1)
            gpsimd.wait_ge(ini, 1)
            nc.gpsimd.affine_select(
                out=ident[:],
                in_=ident[:],
                compare_op=mybir.AluOpType.not_equal,
                fill=1.0,
                base=0,
                pattern=[[-1, P]],
                channel_multiplier=1,
            ).then_inc(ini, 1)

        @block.sync
        def _(sync: bass.BassEngine):
            col = 0
            for c, csz in enumerate(chunk_sizes):
                if c >= NBUF:
                    # buffer c%NBUF free once chunk c-NBUF fully transposed
                    sync.wait_ge(tp, tp_after_chunk[c - NBUF])
                sync.dma_start(
                    chunks[c % NBUF][:, : csz * D],
                    emb[:, col : col + csz * D],
                ).then_inc(lds[c % NBUF], 16)
                col += csz * D
            sync.wait_ge(vr, vr_after_group[-1])
            sync.dma_start(parts[:], partials[:]).then_inc(st, 16)
            sync.wait_ge(st, 16)

        @block.tensor
        def _(tensor: bass.BassEngine):
            tensor.wait_ge(ini, 2)
            prev_chunk = -1
            for G, (c, b, q, t0, g0, k) in enumerate(groups):
                if c != prev_chunk:
                    tensor.wait_ge(lds[b], 16 * (c // NBUF + 1))
                    prev_chunk = c
                if G >= NPSUM:
                    tensor.wait_ge(vr, vr_after_group[G - NPSUM])
                for i in range(k):
                    tl = g0 + i
                    nc.tensor.transpose(
                        out=ph0[q][:, i * P : (i + 1) * P],
                        in_=chunks[b][:, tl * D : tl * D + P],
                        identity=ident[:],
                    ).then_inc(tp, 1)
                    nc.tensor.transpose(
                        out=ph1[q][:, i * P : (i + 1) * P],
                        in_=chunks[b][:, tl * D + P : (tl + 1) * D],
                        identity=ident[:],
                    ).then_inc(tp, 1)

        @block.vector
        def _(vector: bass.BassEngine):
            for G, (c, b, q, t0, g0, k) in enumerate(groups):
                vector.wait_ge(tp, tp_after_group[G])
                nc.vector.reduce_max(
                    partials[:, t0 : t0 + k],
                    ph0[q][:, : k * P].rearrange("p (t r) -> p t r", r=P),
                    axis=mybir.AxisListType.X,
                ).then_inc(vr, 1)
                nc.vector.reduce_max(
                    partials[:, NT + t0 : NT + t0 + k],
                    ph1[q][:, : k * P].rearrange("p (t r) -> p t r", r=P),
                    axis=mybir.AxisListType.X,
                ).then_inc(vr, 1)

    nc.compile()
    return nc


def kernel(embeddings, study_indexes, num_segments):
    from concourse.bass_utils import run_bass_kernel_spmd

    emb = np.ascontiguousarray(np.asarray(embeddings, dtype=np.float32))
    idx = np.asarray(study_indexes).astype(np.int64)
    S = int(num_segments)
    N = emb.shape[0]
    Nc = N // N_CORES
    nt = (Nc + P - 1) // P

    nc = _NC_CACHE.get(nt)
    if nc is None:
        nc = _NC_CACHE[nt] = build_nc(nt)

    plans = []
    in_maps = []
    for c in range(N_CORES):
        idx_c = idx[c * Nc : (c + 1) * Nc]
        shard = emb[c * Nc : (c + 1) * Nc]
        order = np.argsort(idx_c, kind="stable")
        rows = np.empty(nt * P, np.int64)
        rows[:Nc] = order
        rows[Nc:] = order[-1]                      # tail pad: repeat last row
        sorted_vals = shard[rows]                  # [nt*128, 256]
        arr = sorted_vals.reshape(nt, P, D).transpose(1, 0, 2).reshape(P, nt * D)
        seg_sorted = idx_c[rows]
        tile_first = seg_sorted[0::P]
        tile_last = seg_sorted[P - 1 :: P]
        bnd_t = np.nonzero(tile_first != tile_last)[0]
        row_sel = (bnd_t[:, None] * P + np.arange(P)[None, :]).ravel()
        plans.append((seg_sorted, bnd_t, sorted_vals[row_sel]))
        del sorted_vals
        in_maps.append({"emb": np.ascontiguousarray(arr)})

    res = run_bass_kernel_spmd(nc, in_maps, list(range(N_CORES)))
    global _LAST_RESULT
    _LAST_RESULT = res

    out = np.full((S, D), -np.inf, dtype=np.float32)
    for c in range(N_CORES):
        parts = res.results[c]["parts"]            # [128, 2*nt]
        seg_sorted, bnd_t, bvals = plans[c]
        tile_first = seg_sorted[0::P]              # [nt]
        pure = np.ones(nt, bool)
        pure[bnd_t] = False

        # pure tiles: combine device partials by segment run
        pure_t = np.nonzero(pure)[0]
        if len(pure_t):
            psegs = tile_first[pure_t]
            starts = np.concatenate([[0], np.nonzero(np.diff(psegs))[0] + 1])
            p0 = parts[:, pure_t]                  # [128, npure]
            p1 = parts[:, nt + pure_t]
            m0 = np.maximum.reduceat(p0, starts, axis=1)
            m1 = np.maximum.reduceat(p1, starts, axis=1)
            for j, s in enumerate(psegs[starts]):
                np.maximum(out[s, :P], m0[:, j], out=out[s, :P])
                np.maximum(out[s, P:], m1[:, j], out=out[s, P:])

        # boundary tiles: re-reduce from the raw (already sorted) rows
        if len(bnd_t):
            row_sel = (bnd_t[:, None] * P + np.arange(P)[None, :]).ravel()
            bsegs = seg_sorted[row_sel]            # sorted within and across runs
            starts = np.concatenate([[0], np.nonzero(np.diff(bsegs))[0] + 1])
            m = np.maximum.reduceat(bvals, starts, axis=0)
            for j, s in enumerate(bsegs[starts]):
                np.maximum(out[s], m[j], out=out[s])
    return out
